# revision 10
# baseline (speedup 1.0000x reference)
"""Trainium2 Bass kernel for per-sample generated low-rank linear:

    h   = inp @ U                      # [B, 128] -> [B, 32]
    h2  = einsum('bi,bio->bo', h, gen_weight.reshape(B, 32, 32))
    out = h2 @ V + bias                # [B, 32] -> [B, 128]

Strategy: pure data parallel over 8 NeuronCores (B rows split evenly).
Per core, per 128-row tile (b in partitions):
  PE:   h = inpT.T @ U (f32r), transpose(h2), h2T.T @ V plus a K=1
        ones x bias matmul accumulated into the same PSUM tile.
  ACT:  h PSUM->SBUF, h2T PSUM->SBUF, out PSUM->SBUF copies; issues
        inp/out DMAs (HWDGE).
  DVE:  tmp[:, 0:512]    = gw[:, 0:512]    * h_bcast[:, 0:16, :]
        (h broadcast along o via a stride-0 AP dim), then tree adds
        1024 -> 64 (i-major halving keeps the 32 o-lanes aligned).
  Pool: tmp[:, 512:1024] = gw[:, 512:1024] * h_bcast[:, 16:32, :] and
        the final 64 -> 32 tree level.
  SP:   gen_weight DMA issue (HWDGE).

Host-side prep (part of kernel()): shard rows, transpose the inp shard
to [128, BL] (feature-major so the contraction dim is the partition dim
on-chip, 4KB contiguous DMA runs), regroup gen_weight to
[P, NTILES, 1024] (32KB contiguous runs), and un-permute the
[P, NTILES, F] device output layout.
"""

import sys

if "/opt/trn_rl_repo" not in sys.path:
    sys.path.insert(0, "/opt/trn_rl_repo")

import numpy as np

B = 131072
IN_FEAT = 128
OUT_FEAT = 128
RANK = 32
N_CORES = 8
BL = B // N_CORES          # rows per core
P = 128                    # partitions / rows per tile
NTILES = BL // P           # 128 tiles per core
CH = 8                     # tiles per DMA chunk
NCH = NTILES // CH

_cached = {}


def _build_nc():
    from concourse import bacc, masks, mybir
    from concourse.tile import TileContext

    f32 = mybir.dt.float32
    f32r = mybir.dt.float32r
    bf16 = mybir.dt.bfloat16
    Alu = mybir.AluOpType
    RR = RANK * RANK

    nc = bacc.Bacc(None)
    inp_e = nc.declare_dram_parameter("inp", [IN_FEAT, BL], f32r, isOutput=False)
    gw_e = nc.declare_dram_parameter(
        "gen_weight", [P, NTILES, RR], f32, isOutput=False
    )
    u_e = nc.declare_dram_parameter("u", [IN_FEAT, RANK], f32r, isOutput=False)
    v_e = nc.declare_dram_parameter("v", [RANK, OUT_FEAT], f32, isOutput=False)
    bias_e = nc.declare_dram_parameter("bias", [1, OUT_FEAT], f32, isOutput=False)
    out_e = nc.declare_dram_parameter(
        "out", [P, NTILES, OUT_FEAT], f32, isOutput=True
    )

    with TileContext(nc) as tc:
        with (
            tc.tile_pool(name="const", bufs=1) as cpool,
            tc.tile_pool(name="io", bufs=2) as io,
            tc.tile_pool(name="gwp", bufs=2) as gwp,
            tc.tile_pool(name="work", bufs=3) as work,
            tc.tile_pool(name="pH", bufs=2, space="PSUM") as pH,
            tc.tile_pool(name="pS", bufs=2, space="PSUM") as pS,
            tc.tile_pool(name="pO", bufs=2, space="PSUM") as pO,
        ):
            ident = cpool.tile([P, P], bf16)
            masks.make_identity(nc, ident[:])
            u_sb = cpool.tile([IN_FEAT, RANK], f32r)
            nc.sync.dma_start(u_sb[:], u_e[:])
            v_sb = cpool.tile([RANK, OUT_FEAT], bf16)
            nc.gpsimd.dma_start(v_sb[:], v_e[:])  # SWDGE casts f32 -> bf16
            bias_sb = cpool.tile([1, OUT_FEAT], bf16)
            nc.gpsimd.dma_start(bias_sb[:], bias_e[:])
            ones_sb = cpool.tile([1, P], bf16)
            nc.vector.memset(ones_sb[:], 1.0)

            for c in range(NCH):
                inpT = io.tile([P, CH, P], f32r, tag="inpT")
                nc.scalar.dma_start(inpT[:], inp_e[:, c * CH * P : (c + 1) * CH * P])
                gw_c = gwp.tile([P, CH, RR], f32, tag="gw")
                nc.sync.dma_start(gw_c[:], gw_e[:, c * CH : (c + 1) * CH, :])
                out_c = io.tile([P, CH, OUT_FEAT], f32, tag="out")

                for t in range(CH):
                    # h[b, i] (b = partition), then to SBUF so Pool can read
                    hps = pH.tile([P, RANK], f32, tag="hps")
                    nc.tensor.matmul(hps[:], inpT[:, t, :], u_sb[:])
                    h_sb = work.tile([P, RANK], f32, tag="h")
                    nc.scalar.copy(h_sb[:], hps[:])
                    hb = h_sb[:].unsqueeze(-1)  # [P, 32, 1]

                    # tmp[b, 32i+o] = gw[b, 32i+o] * h[b, i]; DVE low half,
                    # Pool high half (writes bf16)
                    tmp = work.tile([P, RR], bf16, tag="tmp")
                    gw_v = gw_c[:, t, :].rearrange("p (i o) -> p i o", i=RANK)
                    tmp_v = tmp[:].rearrange("p (i o) -> p i o", i=RANK)
                    half = RANK // 2
                    nc.vector.tensor_tensor(
                        tmp_v[:, 0:half, :],
                        gw_v[:, 0:half, :],
                        hb[:, 0:half, :].broadcast_to([P, half, RANK]),
                        Alu.mult,
                    )
                    nc.gpsimd.tensor_tensor(
                        tmp_v[:, half:RANK, :],
                        gw_v[:, half:RANK, :],
                        hb[:, half:RANK, :].broadcast_to([P, half, RANK]),
                        Alu.mult,
                    )

                    # tree-add over i (i-major halving keeps o lanes aligned)
                    nc.vector.tensor_tensor(
                        tmp[:, 0:512], tmp[:, 0:512], tmp[:, 512:1024], Alu.add
                    )
                    nc.vector.tensor_tensor(
                        tmp[:, 0:256], tmp[:, 0:256], tmp[:, 256:512], Alu.add
                    )
                    nc.vector.tensor_tensor(
                        tmp[:, 0:128], tmp[:, 0:128], tmp[:, 128:256], Alu.add
                    )
                    nc.vector.tensor_tensor(
                        tmp[:, 0:64], tmp[:, 0:64], tmp[:, 64:128], Alu.add
                    )
                    nc.gpsimd.tensor_tensor(
                        tmp[:, 0:32], tmp[:, 0:32], tmp[:, 32:64], Alu.add
                    )

                    # h2 -> h2.T -> out = h2 @ V + bias
                    psS = pS.tile([RANK, P], bf16, tag="h2T")
                    nc.tensor.transpose(psS[:], tmp[:, 0:RANK], ident[:])
                    h2T = work.tile([RANK, P], bf16, tag="h2T_sb")
                    nc.scalar.copy(h2T[:], psS[:])

                    pso = pO.tile([P, OUT_FEAT], f32, tag="outp")
                    nc.tensor.matmul(pso[:], h2T[:], v_sb[:], start=True, stop=False)
                    nc.tensor.matmul(
                        pso[:], ones_sb[:], bias_sb[:], start=False, stop=True
                    )
                    nc.scalar.copy(out_c[:, t, :], pso[:])

                nc.scalar.dma_start(out_e[:, c * CH : (c + 1) * CH, :], out_c[:])

    nc.compile()
    return nc


def _get_nc():
    if "nc" not in _cached:
        _cached["nc"] = _build_nc()
    return _cached["nc"]


def run(inputs, trace=False):
    """Returns (full_output [B, OUT_FEAT] fp32, BassKernelResults)."""
    from concourse.bass_utils import run_bass_kernel_spmd

    inp = np.ascontiguousarray(inputs["inp"], dtype=np.float32)
    gw = np.ascontiguousarray(inputs["gen_weight"], dtype=np.float32)
    u = np.ascontiguousarray(inputs["U"], dtype=np.float32)
    v = np.ascontiguousarray(inputs["V"], dtype=np.float32)
    bias = np.ascontiguousarray(inputs["bias"], dtype=np.float32)

    in_maps = []
    for i in range(N_CORES):
        sl = slice(i * BL, (i + 1) * BL)
        in_maps.append(
            {
                "inp": np.ascontiguousarray(inp[sl].T),
                "gen_weight": np.ascontiguousarray(
                    gw[sl].reshape(NTILES, P, RANK * RANK).transpose(1, 0, 2)
                ),
                "u": u,
                "v": v,
                "bias": bias.reshape(1, OUT_FEAT),
            }
        )

    nc = _get_nc()
    res = run_bass_kernel_spmd(nc, in_maps, core_ids=list(range(N_CORES)), trace=trace)
    # device layout [P, NTILES, F]: sample s = n*128 + p
    shards = [
        r["out"].transpose(1, 0, 2).reshape(BL, OUT_FEAT) for r in res.results
    ]
    out = np.concatenate(shards, axis=0)
    return out, res


def kernel(**inputs):
    out, _ = run(inputs, trace=False)
    return out


# revision 11
# speedup vs baseline: 1.2988x; 1.2988x over previous
"""Trainium2 Bass kernel for per-sample generated low-rank linear:

    h   = inp @ U                      # [B, 128] -> [B, 32]
    h2  = einsum('bi,bio->bo', h, gen_weight.reshape(B, 32, 32))
    out = h2 @ V + bias                # [B, 32] -> [B, 128]

Strategy: pure data parallel over 8 NeuronCores (B rows split evenly).
Per core, per 128-row tile (b in partitions):
  PE:   h_rep = inpT.T @ U_rep (f32r, U columns each repeated 32x) so
        PSUM holds h_rep[b, 32i+o] = h[b, i]; transpose(h2); h2T.T @ V
        plus a K=1 ones x bias matmul into the same PSUM tile.
  DVE:  flat tmp = gw * h_rep (bf16 out) + tree levels 2-4.
  Pool: tree levels 1 and 5 (SBUF only -- Pool cannot read PSUM).
  ACT:  h2T and out PSUM->SBUF copies; issues inp/out DMAs (HWDGE).
  SP:   gen_weight DMA issue (HWDGE).

Host-side prep (part of kernel()): shard rows, transpose the inp shard
to [128, BL] (feature-major so the contraction dim is the partition dim
on-chip, 4KB contiguous DMA runs), regroup gen_weight to
[P, NTILES, 1024] (32KB contiguous runs), and un-permute the
[P, NTILES, F] device output layout.
"""

import sys

if "/opt/trn_rl_repo" not in sys.path:
    sys.path.insert(0, "/opt/trn_rl_repo")

import numpy as np

B = 131072
IN_FEAT = 128
OUT_FEAT = 128
RANK = 32
N_CORES = 8
BL = B // N_CORES          # rows per core
P = 128                    # partitions / rows per tile
NTILES = BL // P           # 128 tiles per core
CH = 8                     # tiles per DMA chunk
NCH = NTILES // CH

_cached = {}


def _build_nc():
    from concourse import bacc, masks, mybir
    from concourse.tile import TileContext

    f32 = mybir.dt.float32
    f32r = mybir.dt.float32r
    bf16 = mybir.dt.bfloat16
    Alu = mybir.AluOpType
    RR = RANK * RANK

    nc = bacc.Bacc(None)
    inp_e = nc.declare_dram_parameter("inp", [IN_FEAT, BL], f32r, isOutput=False)
    gw_e = nc.declare_dram_parameter(
        "gen_weight", [P, NTILES, RR], f32, isOutput=False
    )
    urep_e = nc.declare_dram_parameter("u_rep", [IN_FEAT, RR], f32r, isOutput=False)
    v_e = nc.declare_dram_parameter("v", [RANK, OUT_FEAT], f32, isOutput=False)
    bias_e = nc.declare_dram_parameter("bias", [1, OUT_FEAT], f32, isOutput=False)
    out_e = nc.declare_dram_parameter(
        "out", [P, NTILES, OUT_FEAT], f32, isOutput=True
    )

    with TileContext(nc) as tc:
        with (
            tc.tile_pool(name="const", bufs=1) as cpool,
            tc.tile_pool(name="io", bufs=2) as io,
            tc.tile_pool(name="gwp", bufs=2) as gwp,
            tc.tile_pool(name="work", bufs=3) as work,
            tc.tile_pool(name="pH", bufs=2, space="PSUM") as pH,
            tc.tile_pool(name="pS", bufs=2, space="PSUM") as pS,
            tc.tile_pool(name="pO", bufs=2, space="PSUM") as pO,
        ):
            ident = cpool.tile([P, P], bf16)
            masks.make_identity(nc, ident[:])
            urep_sb = cpool.tile([IN_FEAT, RR], f32r)
            nc.sync.dma_start(urep_sb[:], urep_e[:])
            v_sb = cpool.tile([RANK, OUT_FEAT], bf16)
            nc.gpsimd.dma_start(v_sb[:], v_e[:])  # SWDGE casts f32 -> bf16
            bias_sb = cpool.tile([1, OUT_FEAT], bf16)
            nc.gpsimd.dma_start(bias_sb[:], bias_e[:])
            ones_sb = cpool.tile([1, P], bf16)
            nc.vector.memset(ones_sb[:], 1.0)

            for c in range(NCH):
                inpT = io.tile([P, CH, P], f32r, tag="inpT")
                nc.scalar.dma_start(inpT[:], inp_e[:, c * CH * P : (c + 1) * CH * P])
                gw_c = gwp.tile([P, CH, RR], f32, tag="gw")
                nc.sync.dma_start(gw_c[:], gw_e[:, c * CH : (c + 1) * CH, :])
                out_c = io.tile([P, CH, OUT_FEAT], f32, tag="out")

                for t in range(CH):
                    # h_rep[b, 32i+o] = h[b, i]  (b = partition)
                    hrep = pH.tile([P, RR], f32, tag="hrep")
                    nc.tensor.matmul(hrep[:, 0:512], inpT[:, t, :], urep_sb[:, 0:512])
                    nc.tensor.matmul(
                        hrep[:, 512:1024], inpT[:, t, :], urep_sb[:, 512:1024]
                    )

                    # tmp = gw * h_rep; tree-add over i (i-major halving
                    # keeps the 32 o-lanes aligned)
                    tmp = work.tile([P, RR], bf16, tag="tmp")
                    nc.vector.tensor_tensor(tmp[:], gw_c[:, t, :], hrep[:], Alu.mult)
                    nc.gpsimd.tensor_tensor(
                        tmp[:, 0:512], tmp[:, 0:512], tmp[:, 512:1024], Alu.add
                    )
                    nc.vector.tensor_tensor(
                        tmp[:, 0:256], tmp[:, 0:256], tmp[:, 256:512], Alu.add
                    )
                    nc.vector.tensor_tensor(
                        tmp[:, 0:128], tmp[:, 0:128], tmp[:, 128:256], Alu.add
                    )
                    nc.vector.tensor_tensor(
                        tmp[:, 0:64], tmp[:, 0:64], tmp[:, 64:128], Alu.add
                    )
                    nc.gpsimd.tensor_tensor(
                        tmp[:, 0:32], tmp[:, 0:32], tmp[:, 32:64], Alu.add
                    )

                    # h2 -> h2.T -> out = h2 @ V + bias
                    psS = pS.tile([RANK, P], bf16, tag="h2T")
                    nc.tensor.transpose(psS[:], tmp[:, 0:RANK], ident[:])
                    h2T = work.tile([RANK, P], bf16, tag="h2T_sb")
                    nc.scalar.copy(h2T[:], psS[:])

                    pso = pO.tile([P, OUT_FEAT], f32, tag="outp")
                    nc.tensor.matmul(pso[:], h2T[:], v_sb[:], start=True, stop=False)
                    nc.tensor.matmul(
                        pso[:], ones_sb[:], bias_sb[:], start=False, stop=True
                    )
                    nc.scalar.copy(out_c[:, t, :], pso[:])

                nc.scalar.dma_start(out_e[:, c * CH : (c + 1) * CH, :], out_c[:])

    nc.compile()
    return nc


def _get_nc():
    if "nc" not in _cached:
        _cached["nc"] = _build_nc()
    return _cached["nc"]


def run(inputs, trace=False):
    """Returns (full_output [B, OUT_FEAT] fp32, BassKernelResults)."""
    from concourse.bass_utils import run_bass_kernel_spmd

    inp = np.ascontiguousarray(inputs["inp"], dtype=np.float32)
    gw = np.ascontiguousarray(inputs["gen_weight"], dtype=np.float32)
    u = np.ascontiguousarray(inputs["U"], dtype=np.float32)
    v = np.ascontiguousarray(inputs["V"], dtype=np.float32)
    bias = np.ascontiguousarray(inputs["bias"], dtype=np.float32)

    in_maps = []
    for i in range(N_CORES):
        sl = slice(i * BL, (i + 1) * BL)
        in_maps.append(
            {
                "inp": np.ascontiguousarray(inp[sl].T),
                "gen_weight": np.ascontiguousarray(
                    gw[sl].reshape(NTILES, P, RANK * RANK).transpose(1, 0, 2)
                ),
                "u_rep": np.repeat(u, RANK, axis=1),
                "v": v,
                "bias": bias.reshape(1, OUT_FEAT),
            }
        )

    nc = _get_nc()
    res = run_bass_kernel_spmd(nc, in_maps, core_ids=list(range(N_CORES)), trace=trace)
    # device layout [P, NTILES, F]: sample s = n*128 + p
    shards = [
        r["out"].transpose(1, 0, 2).reshape(BL, OUT_FEAT) for r in res.results
    ]
    out = np.concatenate(shards, axis=0)
    return out, res


def kernel(**inputs):
    out, _ = run(inputs, trace=False)
    return out


# revision 14
# speedup vs baseline: 1.3250x; 1.0202x over previous
"""Trainium2 Bass kernel for per-sample generated low-rank linear:

    h   = inp @ U                      # [B, 128] -> [B, 32]
    h2  = einsum('bi,bio->bo', h, gen_weight.reshape(B, 32, 32))
    out = h2 @ V + bias                # [B, 32] -> [B, 128]

Strategy: pure data parallel over 8 NeuronCores (B rows split evenly).
gen_weight is staged o-major (gw2[b, 32*o+i] = gw[b, 32*i+o]) so the
per-sample GEMV becomes, per 128-row tile (b in partitions):
  PE:   h = inpT.T @ U (one tiny N=32 matmul into PSUM).
  DVE:  tmp[b,o,i] = gw2[b,o,i] * h[b,i]  (h broadcast on the middle
        dim -- all APs innermost-contiguous) for o in [12,32); then
        h2 = reduce_add(tmp, innermost i) in ONE 3D tensor_reduce.
  Pool: the o in [0,12) slice of the multiply (contiguous low chunk).
  PE:   4 tiles' h2 are collected in a quad tile, transposed once, and
        multiplied by a block-diagonal V (plus a K=1 ones x bias_rep
        matmul) -> all 4 tiles' outputs in one [128, 512] PSUM bank.
  ACT:  quad transpose + out4 PSUM->SBUF copies; issues inp/out DMAs.
  SP:   gen_weight DMA issue (HWDGE); gw chunks alternate SP/ACT rings.

Host-side prep (part of kernel()): shard rows, transpose the inp shard
to [128, BL] (feature-major: contraction dim = partition dim on-chip),
regroup gen_weight to [P, NTILES, 1024] o-major (32KB contiguous DMA
runs per partition), build the block-diagonal V and replicated bias,
and un-permute the [P, NTILES, F] device output layout.
"""

import sys

if "/opt/trn_rl_repo" not in sys.path:
    sys.path.insert(0, "/opt/trn_rl_repo")

import numpy as np

B = 131072
IN_FEAT = 128
OUT_FEAT = 128
RANK = 32
N_CORES = 8
BL = B // N_CORES          # rows per core
P = 128                    # partitions / rows per tile
NTILES = BL // P           # 128 tiles per core
CH = 8                     # tiles per DMA chunk
NCH = NTILES // CH
QD = 4                     # tiles per output quad
OSPLIT = 12                # o in [0,OSPLIT) multiplied on Pool, rest on DVE

_cached = {}


def _build_nc():
    from concourse import bacc, masks, mybir
    from concourse.tile import TileContext

    f32 = mybir.dt.float32
    f32r = mybir.dt.float32r
    bf16 = mybir.dt.bfloat16
    Alu = mybir.AluOpType
    RR = RANK * RANK

    nc = bacc.Bacc(None)
    inp_e = nc.declare_dram_parameter("inp", [IN_FEAT, BL], f32r, isOutput=False)
    gw_e = nc.declare_dram_parameter(
        "gen_weight", [P, NTILES, RR], f32, isOutput=False
    )
    u_e = nc.declare_dram_parameter("u", [IN_FEAT, RANK], f32r, isOutput=False)
    vblk_e = nc.declare_dram_parameter(
        "v_blk", [QD * RANK, QD * OUT_FEAT], f32, isOutput=False
    )
    biasr_e = nc.declare_dram_parameter(
        "bias_rep", [1, QD * OUT_FEAT], f32, isOutput=False
    )
    out_e = nc.declare_dram_parameter(
        "out", [P, NTILES, OUT_FEAT], f32, isOutput=True
    )

    with TileContext(nc) as tc:
        with (
            tc.tile_pool(name="const", bufs=1) as cpool,
            tc.tile_pool(name="io", bufs=2) as io,
            tc.tile_pool(name="gwp", bufs=3) as gwp,
            tc.tile_pool(name="work", bufs=3) as work,
            tc.tile_pool(name="quad", bufs=2) as quad,
            tc.tile_pool(name="pH", bufs=2, space="PSUM") as pH,
            tc.tile_pool(name="pS", bufs=2, space="PSUM") as pS,
            tc.tile_pool(name="pO", bufs=2, space="PSUM") as pO,
        ):
            ident = cpool.tile([P, P], f32)
            masks.make_identity(nc, ident[:])
            u_sb = cpool.tile([IN_FEAT, RANK], f32r)
            nc.sync.dma_start(u_sb[:], u_e[:])
            vblk_sb = cpool.tile([QD * RANK, QD * OUT_FEAT], bf16)
            nc.gpsimd.dma_start(vblk_sb[:], vblk_e[:])  # SWDGE casts to bf16
            biasr_sb = cpool.tile([1, QD * OUT_FEAT], bf16)
            nc.gpsimd.dma_start(biasr_sb[:], biasr_e[:])
            ones_sb = cpool.tile([1, P], bf16)
            nc.vector.memset(ones_sb[:], 1.0)

            for c in range(NCH):
                inpT = io.tile([P, CH, P], f32r, tag="inpT")
                nc.scalar.dma_start(inpT[:], inp_e[:, c * CH * P : (c + 1) * CH * P])
                gw_c = gwp.tile([P, CH, RR], f32, tag="gw")
                eng = nc.sync if (c % 2 == 0) else nc.scalar
                eng.dma_start(gw_c[:], gw_e[:, c * CH : (c + 1) * CH, :])
                out_c = io.tile([P, CH, OUT_FEAT], f32, tag="out")

                for q in range(CH // QD):
                    h2q = quad.tile([P, QD * RANK], f32, tag="h2q")
                    for tq in range(QD):
                        t = q * QD + tq
                        # h[b, i] (b = partition)
                        hps = pH.tile([P, RANK], f32, tag="hps")
                        nc.tensor.matmul(hps[:], inpT[:, t, :], u_sb[:])
                        h_sb = work.tile([P, RANK], f32, tag="h")
                        nc.scalar.copy(h_sb[:], hps[:])
                        hbc = h_sb[:].unsqueeze(1)  # [P, 1, RANK]

                        # tmp[b, o, i] = gw2[b, o, i] * h[b, i]
                        tmp = work.tile([P, RANK, RANK], bf16, tag="tmp")
                        gw_v = gw_c[:, t, :].rearrange(
                            "p (o i) -> p o i", o=RANK
                        )
                        nc.gpsimd.tensor_tensor(
                            tmp[:, 0:OSPLIT, :],
                            gw_v[:, 0:OSPLIT, :],
                            hbc.broadcast_to([P, OSPLIT, RANK]),
                            Alu.mult,
                        )
                        nc.vector.tensor_tensor(
                            tmp[:, OSPLIT:RANK, :],
                            gw_v[:, OSPLIT:RANK, :],
                            hbc.broadcast_to([P, RANK - OSPLIT, RANK]),
                            Alu.mult,
                        )

                        # h2[b, o] = sum_i tmp[b, o, i] -- one 3D reduce
                        nc.vector.tensor_reduce(
                            h2q[:, tq * RANK : (tq + 1) * RANK],
                            tmp[:],
                            mybir.AxisListType.X,
                            Alu.add,
                        )

                    # quad: transpose 4 tiles' h2 at once, one block-diag
                    # V matmul + ones x bias_rep -> 4 tiles' outputs
                    psQ = pS.tile([QD * RANK, P], f32, tag="qT")
                    nc.tensor.transpose(psQ[:], h2q[:], ident[:])
                    qT = quad.tile([QD * RANK, P], bf16, tag="qT_sb")
                    nc.scalar.copy(qT[:], psQ[:])

                    out4 = pO.tile([P, QD * OUT_FEAT], f32, tag="out4")
                    nc.tensor.matmul(out4[:], qT[:], vblk_sb[:], start=True, stop=False)
                    nc.tensor.matmul(
                        out4[:], ones_sb[:], biasr_sb[:], start=False, stop=True
                    )
                    nc.scalar.copy(
                        out_c[:, q * QD : (q + 1) * QD, :].rearrange(
                            "p t o -> p (t o)"
                        ),
                        out4[:],
                    )

                nc.scalar.dma_start(out_e[:, c * CH : (c + 1) * CH, :], out_c[:])

    nc.compile()
    return nc


def _get_nc():
    if "nc" not in _cached:
        _cached["nc"] = _build_nc()
    return _cached["nc"]


def run(inputs, trace=False):
    """Returns (full_output [B, OUT_FEAT] fp32, BassKernelResults)."""
    from concourse.bass_utils import run_bass_kernel_spmd

    inp = np.ascontiguousarray(inputs["inp"], dtype=np.float32)
    gw = np.ascontiguousarray(inputs["gen_weight"], dtype=np.float32)
    u = np.ascontiguousarray(inputs["U"], dtype=np.float32)
    v = np.ascontiguousarray(inputs["V"], dtype=np.float32)
    bias = np.ascontiguousarray(inputs["bias"], dtype=np.float32)

    v_blk = np.zeros((QD * RANK, QD * OUT_FEAT), dtype=np.float32)
    for qd in range(QD):
        v_blk[qd * RANK : (qd + 1) * RANK, qd * OUT_FEAT : (qd + 1) * OUT_FEAT] = v
    bias_rep = np.tile(bias.reshape(1, OUT_FEAT), (1, QD))

    in_maps = []
    for i in range(N_CORES):
        sl = slice(i * BL, (i + 1) * BL)
        # o-major regroup: gw2[p, n, 32*o+i] = gw[n*128+p, 32*i+o]
        g = gw[sl].reshape(NTILES, P, RANK, RANK)
        g2 = np.ascontiguousarray(g.transpose(1, 0, 3, 2)).reshape(
            P, NTILES, RANK * RANK
        )
        in_maps.append(
            {
                "inp": np.ascontiguousarray(inp[sl].T),
                "gen_weight": g2,
                "u": u,
                "v_blk": v_blk,
                "bias_rep": bias_rep,
            }
        )

    nc = _get_nc()
    res = run_bass_kernel_spmd(nc, in_maps, core_ids=list(range(N_CORES)), trace=trace)
    # device layout [P, NTILES, F]: sample s = n*128 + p
    shards = [
        r["out"].transpose(1, 0, 2).reshape(BL, OUT_FEAT) for r in res.results
    ]
    out = np.concatenate(shards, axis=0)
    return out, res


def kernel(**inputs):
    out, _ = run(inputs, trace=False)
    return out
